# revision 25
# baseline (speedup 1.0000x reference)
"""Causal self-attention on 8 trn2 NeuronCores.

Sharding (per the batch+head hint): core c handles batch b = c//2 (data
parallel) and head-group g = c%2 (8 of 16 heads; tensor-parallel slice of
w_qkv columns / w_out rows). Each core computes a full-batch-slice partial
of the output projection over its 512 head dims; the two partials per batch
are summed on gather (the "all-reduce after out_proj").

v2: fully interleaved single-pass emission. The attention inner loop is
scalar(exp)-bound (~650ns/chunk vs ~480ns PE), so qkv matmuls of the next
512-token block and out-proj matmuls of the previous block are woven between
attention chunks to keep the PE saturated. x arrives pre-transposed from the
host (no PE transposes); v is computed in natural [token, dim] orientation
directly (no transposes); weights are packed host-side in consumption order
so compute starts ~4us in; causal masking is a multiplicative bf16 triangle
applied to probs post-exp (2x DVE rate, no PSUM read-modify-write); output
is bf16 (partials summed on host in fp32).
"""
import numpy as np
from collections import deque

B = 4
S = 2048
D = 1024
HG = 8           # heads per core
DH = 64
NCORES = 8
NB = S // 512    # 512-token windows
KC = D // 128    # contraction chunks over D

_CACHE = {}


class _XTView:
    """xs[(tb, ki)] -> ki-th 512-col slice of the per-block xT tile."""

    def __init__(self, blocks):
        self.blocks = blocks

    def __getitem__(self, key):
        tb, ki = key
        return self.blocks[tb][:, ki * 512:(ki + 1) * 512]


def _build_nc(dump=False):
    import concourse.bass as bass  # noqa
    import concourse.mybir as mybir
    import concourse.tile as tile
    from concourse import bacc

    F32 = mybir.dt.float32
    BF = mybir.dt.bfloat16
    Exp = mybir.ActivationFunctionType.Exp

    nc = bacc.Bacc("TRN2", target_bir_lowering=False, debug=False,
                   enable_asserts=False, num_devices=NCORES)
    if dump:
        dumps = {nm: nc.dram_tensor(nm + "_dump", shp, BF,
                                    kind="ExternalOutput")
                 for nm, shp in [("qT", [128, 4 * S]), ("kT", [128, 4 * S]),
                                 ("v1", [128, HG * 16 * 65]),
                                 ("oT", [128, 4 * S])]}
    xT_d = nc.dram_tensor("xT", [D, S], BF, kind="ExternalInput")
    wqk_d = nc.dram_tensor("wqk", [128, 64 * 128], BF, kind="ExternalInput")
    wv_d = nc.dram_tensor("wv", [128, KC * 512], BF, kind="ExternalInput")
    wo_d = nc.dram_tensor("wo", [128, 4 * D], BF, kind="ExternalInput")
    tri_d = nc.dram_tensor("tri", [128, 128], BF, kind="ExternalInput")
    out_d = nc.dram_tensor("out", [S, D], BF, kind="ExternalOutput")

    with tile.TileContext(nc) as tc:
        with tc.tile_pool(name="persist", bufs=1) as persist, \
             tc.tile_pool(name="xT", bufs=2) as xT_pool, \
             tc.tile_pool(name="probs", bufs=20) as pr_pool, \
             tc.tile_pool(name="recip", bufs=2) as rc_pool, \
             tc.tile_pool(name="rbc", bufs=2) as rb_pool, \
             tc.tile_pool(name="obig", bufs=1) as obig_pool, \
             tc.tile_pool(name="ostage", bufs=3) as ost_pool, \
             tc.tile_pool(name="ps_sc", bufs=2, space="PSUM") as ps_sc, \
             tc.tile_pool(name="ps_out", bufs=2, space="PSUM") as ps_out, \
             tc.tile_pool(name="ps_mm", bufs=2, space="PSUM") as ps_mm:
            qT = persist.tile([128, 4 * S], BF)
            kT = persist.tile([128, 4 * S], BF)
            v1 = persist.tile([128, HG * 16 * 65], BF)
            oT = obig_pool.tile([128, 4 * S], BF)
            # separate tiles per weight block so consumers wait only on
            # their own DMA (fine-grained startup)
            wqk_m = [persist.tile([128, 1024], BF, name=f"wqk_{m}")
                     for m in range(8)]
            wv_k = [persist.tile([128, 512], BF, name=f"wv_{ki}")
                    for ki in range(KC)]
            wo_sb = persist.tile([128, 4 * D], BF)
            tri01 = persist.tile([128, 128], BF)
            ones128 = persist.tile([128, 128], F32)
            nc.gpsimd.memset(ones128[:], 1.0)
            # ones column of every [*, 65] v chunk (softmax denominator row)
            nc.scalar.copy(
                v1[:].rearrange("p (c u) -> p c u", u=65)[:, :, 64:65],
                ones128[:].rearrange("p (c u) -> p c u", u=1),
            )

            xT_blk = {}

            def dma_xT(tb, split=False):
                t = xT_pool.tile([128, KC * 512], BF, tag="xT",
                                 name=f"xT_{tb}")
                src = xT_d.ap()[:, tb * 512:(tb + 1) * 512] \
                    .rearrange("(k p) s -> p k s", p=128)
                dst = t[:].rearrange("p (k s) -> p k s", k=KC)
                if split:
                    nc.sync.dma_start(dst[:, :4], src[:, :4])
                    nc.sync.dma_start(dst[:, 4:], src[:, 4:])
                else:
                    nc.sync.dma_start(dst, src)
                xT_blk[tb] = t

            xT_tiles = _XTView(xT_blk)

            # DMAs in strict consumption order: the first qkv granule needs
            # only wqk block 0 + the first xT chunks (~0.9MB), so compute
            # starts ~3us in; everything else streams behind it.
            nc.sync.dma_start(wqk_m[0][:], wqk_d[:, 0:1024])
            dma_xT(0, split=True)
            for m in range(1, 8):
                nc.sync.dma_start(wqk_m[m][:],
                                  wqk_d[:, m * 1024:(m + 1) * 1024])
            nc.sync.dma_start(tri01[:], tri_d.ap())
            for ki in range(KC):
                nc.sync.dma_start(wv_k[ki][:],
                                  wv_d[:, ki * 512:(ki + 1) * 512])

            def qkv_granules(tb):
                """~0.9us PE granules: q/k m-blocks (2 halves each) and
                natural-orientation v token-blocks (2 halves each)."""
                gs = []
                xs = xT_tiles
                for m in range(8):
                    cell = {}

                    def g1(tb=tb, m=m, cell=cell):
                        ps = ps_mm.tile([128, 512], F32, tag="mm",
                                        name=f"qk_{tb}_{m}")
                        cell["ps"] = ps
                        for ki in range(4):
                            nc.tensor.matmul(
                                ps[:],
                                wqk_m[m][:, ki * 128:(ki + 1) * 128],
                                xs[(tb, ki)][:],
                                start=(ki == 0), stop=False)

                    def g2(tb=tb, m=m, cell=cell):
                        ps = cell["ps"]
                        for ki in range(4, KC):
                            nc.tensor.matmul(
                                ps[:],
                                wqk_m[m][:, ki * 128:(ki + 1) * 128],
                                xs[(tb, ki)][:],
                                start=False, stop=(ki == KC - 1))
                        dst = qT if m < 4 else kT
                        mm = m if m < 4 else m - 4
                        nc.vector.tensor_copy(
                            dst[:, mm * S + tb * 512:mm * S + tb * 512 + 512],
                            ps[:])
                    gs.append(g1)
                    gs.append(g2)
                for t4 in range(4):
                    cell = {}

                    def gv1(tb=tb, t4=t4, cell=cell):
                        ps = ps_mm.tile([128, 512], F32, tag="mm",
                                        name=f"v_{tb}_{t4}")
                        cell["ps"] = ps
                        for ki in range(4):
                            nc.tensor.matmul(
                                ps[:],
                                xs[(tb, ki)][:, t4 * 128:t4 * 128 + 128],
                                wv_k[ki][:],
                                start=(ki == 0), stop=False)

                    def gv2(tb=tb, t4=t4, cell=cell):
                        ps = cell["ps"]
                        for ki in range(4, KC):
                            nc.tensor.matmul(
                                ps[:],
                                xs[(tb, ki)][:, t4 * 128:t4 * 128 + 128],
                                wv_k[ki][:],
                                start=False, stop=(ki == KC - 1))
                        sck = tb * 4 + t4
                        nc.vector.tensor_copy(
                            v1[:].rearrange("p (h u) -> p h u",
                                            u=16 * 65)[:, :,
                                                       sck * 65:sck * 65 + 64],
                            ps[:].rearrange("p (h u) -> p h u", u=64))
                    gs.append(gv1)
                    gs.append(gv2)
                return gs

            def proj_granules(tb):
                gs = []
                for mt in range(4 * tb, 4 * tb + 4):
                    for half in range(2):
                        cell = {}

                        def g1(mt=mt, half=half, cell=cell):
                            ps = ps_mm.tile([128, 512], F32, tag="mm",
                                            name=f"pj_{mt}_{half}")
                            cell["ps"] = ps
                            for k in range(2):
                                nc.tensor.matmul(
                                    ps[:],
                                    oT[:, k * S + mt * 128:
                                       k * S + mt * 128 + 128],
                                    wo_sb[:, k * D + half * 512:
                                          k * D + half * 512 + 512],
                                    start=(k == 0), stop=False)

                        def g2(mt=mt, half=half, cell=cell):
                            ps = cell["ps"]
                            for k in range(2, 4):
                                nc.tensor.matmul(
                                    ps[:],
                                    oT[:, k * S + mt * 128:
                                       k * S + mt * 128 + 128],
                                    wo_sb[:, k * D + half * 512:
                                          k * D + half * 512 + 512],
                                    start=False, stop=(k == 3))
                            ost = ost_pool.tile([128, 512], BF, tag="ost",
                                                name=f"ost_{mt}_{half}")
                            nc.vector.tensor_copy(ost[:], ps[:])
                            nc.sync.dma_start(
                                out_d[mt * 128:(mt + 1) * 128,
                                      half * 512:(half + 1) * 512],
                                ost[:])
                        gs.append(g1)
                        gs.append(g2)
                return gs

            def att_stream(tb):
                """Yields once per head-pair chunk; exact-causal flash
                attention, transposed orientation, query window tb. Both
                heads of pair hp share kT/qT block r=hp; their scores land
                in halves of one 2-bank psum tile and one wide exp covers
                both (halved scalar instruction overhead)."""
                nch = 4 * tb + 4
                pairs = [(hp, s) for hp in range(4) for s in range(nch)]
                out_ps = {}
                pr_tiles = {}
                LA = 2

                def emit_sc(pp):
                    hp, s = pp
                    r = hp
                    lo = max(128 * s - 512 * tb, 0)
                    n = 512 - lo
                    scp = ps_sc.tile([128, 1024], F32, tag="sc",
                                     name=f"sc_{tb}_{hp}_{s}")
                    for side in range(2):
                        po = 64 * side
                        nc.tensor.matmul(
                            scp[:, side * 512:side * 512 + n],
                            kT[po:po + 64,
                               r * S + s * 128:r * S + s * 128 + 128],
                            qT[po:po + 64,
                               r * S + 512 * tb + lo:r * S + 512 * (tb + 1)],
                            start=True, stop=True)
                    pr = pr_pool.tile([128, 1024], BF, tag="pr",
                                      name=f"pr_{tb}_{hp}_{s}")
                    if lo >= 256:
                        nc.scalar.activation(pr[:, :n], scp[:, :n], Exp)
                        nc.scalar.activation(pr[:, 512:512 + n],
                                             scp[:, 512:512 + n], Exp)
                    else:
                        # one wide exp spanning both halves (middle cols are
                        # unread garbage when lo > 0)
                        nc.scalar.activation(pr[:, :512 + n],
                                             scp[:, :512 + n], Exp)
                    if 128 * s >= 512 * tb:
                        # diagonal: zero the upper triangle of each head's
                        # first 128 probs columns (bf16 mask, 2x DVE)
                        nc.vector.tensor_mul(pr[:, :128], pr[:, :128],
                                             tri01[:])
                        nc.vector.tensor_mul(pr[:, 512:640], pr[:, 512:640],
                                             tri01[:])
                    pr_tiles[pp] = pr

                def emit_av(pp, side):
                    """AV for one head of the pair. Side 0 lags scores by LA
                    pairs; side 1 by a whole row (keeps only 2 out_ps psum
                    tiles live at once)."""
                    hp, s = pp
                    r = hp
                    lo = max(128 * s - 512 * tb, 0)
                    n = 512 - lo
                    h = 2 * hp + side
                    po = 64 * side
                    pr = pr_tiles[pp] if side == 0 else pr_tiles.pop(pp)
                    if s == 0:
                        out_ps[h] = ps_out.tile([65, 512], F32, tag="o",
                                                name=f"ops_{tb}_{h}")
                    nc.tensor.matmul(
                        out_ps[h][:, lo:512],
                        v1[:, (h * 16 + s) * 65:(h * 16 + s) * 65 + 65],
                        pr[:, side * 512:side * 512 + n],
                        start=(s == 0), stop=(s == nch - 1))
                    if s == nch - 1:
                        # evict the whole accumulator to SBUF in one copy so
                        # the psum bank frees before the normalize chain runs
                        op = out_ps.pop(h)
                        den = rc_pool.tile([1, 512], F32, tag="den")
                        nc.vector.tensor_copy(den[:], op[64:65, :])
                        o_sb = rb_pool.tile([64, 512], F32, tag="osb")
                        nc.vector.tensor_copy(o_sb[:], op[0:64, :])
                        rc = rc_pool.tile([1, 512], F32, tag="rc")
                        nc.vector.reciprocal_approx_fast(rc[:], den[:])
                        rb = rb_pool.tile([64, 512], F32, tag="rb")
                        nc.gpsimd.partition_broadcast(rb[:], rc[:])
                        nc.vector.tensor_mul(
                            oT[po:po + 64,
                               r * S + 512 * tb:r * S + 512 * tb + 512],
                            o_sb[:], rb[:])

                for i, pp in enumerate(pairs):
                    emit_sc(pp)
                    if i >= LA:
                        emit_av(pairs[i - LA], 0)
                    if i >= nch:
                        emit_av(pairs[i - nch], 1)
                    yield
                for pp in pairs[-LA:]:
                    emit_av(pp, 0)
                    yield
                for pp in pairs[-nch:]:
                    emit_av(pp, 1)
                    yield

            fillers = deque()

            def drain(k):
                while k and fillers:
                    fillers.popleft()()
                    k -= 1

            for g in qkv_granules(0):
                g()
            nc.sync.dma_start(wo_sb[:], wo_d.ap())
            for tb in range(4):
                if tb < 3:
                    dma_xT(tb + 1)
                    fillers.extend(qkv_granules(tb + 1))
                if tb >= 1:
                    fillers.extend(proj_granules(tb - 1))
                for _ in att_stream(tb):
                    drain(1)
                if tb < 3:
                    drain(len(fillers))
            drain(len(fillers))
            for g in proj_granules(3):
                g()
            if dump:
                for nm, t in [("qT", qT), ("kT", kT), ("v1", v1), ("oT", oT)]:
                    nc.sync.dma_start(dumps[nm].ap(), t[:])
    nc.compile()
    return nc


def _build_nc_debug():
    return _build_nc(dump=True)


def _pack_rows(w, blocks):
    """[blocks*128, n] -> [128, blocks*n] with block-major free dim."""
    n = w.shape[1]
    return np.ascontiguousarray(
        w.reshape(blocks, 128, n).transpose(1, 0, 2).reshape(128, blocks * n))


def _make_in_maps(x, w_qkv, w_out):
    import ml_dtypes
    bf = ml_dtypes.bfloat16
    scale = np.float32(DH ** -0.5)
    k = np.arange(128)[:, None]
    j = np.arange(128)[None, :]
    tri01 = (j >= k).astype(bf)
    in_maps = []
    for c in range(NCORES):
        g = c % 2
        wq = w_qkv[:, g * 512:(g + 1) * 512] * scale
        wk = w_qkv[:, D + g * 512:D + (g + 1) * 512]
        wv = w_qkv[:, 2 * D + g * 512:2 * D + (g + 1) * 512]
        # wqk packed m-major: block (m*8 + ki) = W[ki*128:.., m*128:..]
        W = np.concatenate([wq, wk], axis=1).astype(bf)  # [1024, 1024]
        wqk = np.ascontiguousarray(
            W.reshape(KC, 128, 8, 128).transpose(1, 2, 0, 3)
            .reshape(128, 64 * 128))
        in_maps.append({
            "xT": np.ascontiguousarray(x[c // 2].T).astype(bf),
            "wqk": wqk,
            "wv": _pack_rows(wv.astype(bf), KC),
            "wo": _pack_rows(w_out[g * 512:(g + 1) * 512, :].astype(bf), 4),
            "tri": tri01,
        })
    return in_maps


def kernel(x, w_qkv, w_out):
    from concourse.bass_utils import run_bass_kernel_spmd

    x = np.asarray(x, dtype=np.float32)
    w_qkv = np.asarray(w_qkv, dtype=np.float32)
    w_out = np.asarray(w_out, dtype=np.float32)
    assert x.shape == (B, S, D) and w_qkv.shape == (D, 3 * D) and w_out.shape == (D, D)

    if "nc" not in _CACHE:
        _CACHE["nc"] = _build_nc()
    nc = _CACHE["nc"]

    in_maps = _make_in_maps(x, w_qkv, w_out)
    res = run_bass_kernel_spmd(nc, in_maps, core_ids=list(range(NCORES)),
                               trace=False)
    out = np.empty((B, S, D), dtype=np.float32)
    for b in range(B):
        out[b] = (res.results[2 * b]["out"].astype(np.float32)
                  + res.results[2 * b + 1]["out"].astype(np.float32))
    return out
